# revision 70
# baseline (speedup 1.0000x reference)
"""AttentionPool TRN2 kernel.

Problem: B=2048, S=512, D=128, H=4, T=8 (Q = T*H = 32), C=64.
  k = keys @ Wk^T ; v = keys @ Wv^T
  q = q_flat + (ctx @ Wc^T + bc).reshape(B, Q, D)
  attn = (q @ k^T) * scale * inv_t[q] - slopes[q] * games_ago[s]
  out  = softmax_masked(attn) @ v            -> [B, T, H*D]

Restructured so `keys` is touched by exactly two matmuls per row:
  logits[q,s] = qk'[q,:]  . keys[s,:]        (qk' = (q @ Wk) * scale*inv_t, host-folded)
  pooled[q,:] = (w[q,:] @ keys) @ Wv^T
ALiBi: on unmasked positions games_ago = n_real-1-s, so
  -slope*(n-1-s) = slope*s - slope*(n-1).  slope*s rides into the logits
  as a rank-2 PE matmul (svals x slope + ones x -MASK_NEG), the mask adds
  +MASK_NEG on unmasked positions via a second matmul, and -slope*(n-1)
  is the per-(row,q) softmax shift fed to the exp activation bias (no
  max-reduction needed: shifted logits are <= O(1), masked ones ~ -256).

Memory path: one p-major keys DMA per 4-row group (2 KiB descriptors),
f32->f16 cast split Act/DVE row-granular (gpsimd casts are ~5 ns/col --
kept off that path), mask DMA+cast batched 8 groups with the rank-2
ALiBi bias rows folded into the same PE matmul, fp16 exp/wt/pkt/pooled.

Sharding: pure data parallel over batch, 256 rows/core on 8 cores.
"""

import sys

if "/opt/trn_rl_repo" not in sys.path:
    sys.path.insert(0, "/opt/trn_rl_repo")

import numpy as np

import concourse.bacc as bacc
import concourse.bass as bass
import concourse.tile as tile
from concourse import mybir
from concourse.bass_utils import run_bass_kernel_spmd

B, S, D, H, T, C = 2048, 512, 128, 4, 8, 64
Q = T * H  # 32
N_CORES = 8
ROWS = B // N_CORES  # 256 rows per core
GRP = 4  # batch rows per group -> 4*32 = 128 partitions
MB = 8  # groups per mask batch (32 rows)
BLK = 128  # rows per block (ctx/QKT staging)
SC = 64.0  # power-of-two prescale keeping fp16 operands in normal range
MASK_NEG = 16384.0  # fp16/f32-exact; /SC = 256 pushes masked logits below -126

F32 = mybir.dt.float32
F16 = mybir.dt.float16
U8 = mybir.dt.uint8

NC = S // 128  # 4 chunks of 128 seq positions


def _emit(nc, tc, rows):
    """Emit the per-core program for `rows` batch rows (rows % GRP == 0)."""
    keys_d = nc.declare_dram_parameter("keys", [rows, S, D], F32, isOutput=False)
    mask_d = nc.declare_dram_parameter("mask", [rows, S], U8, isOutput=False)
    ctx_d = nc.declare_dram_parameter("ctx", [rows, C], F32, isOutput=False)
    maug_d = nc.declare_dram_parameter("maug", [C + 1, Q, D], F16, isOutput=False)
    wvt_d = nc.declare_dram_parameter("wvt", [D, D], F16, isOutput=False)
    MP = MB * GRP + 2  # mask-batch partitions incl. svals+ones const rows
    mstat_d = nc.declare_dram_parameter("mstat", [MP, MB, 128], F16,
                                        isOutput=False)
    svals_d = nc.declare_dram_parameter("svals", [2, S], F16, isOutput=False)
    etile_d = nc.declare_dram_parameter("etile", [MB * GRP, MB, 128], F16,
                                        isOutput=False)
    id16_d = nc.declare_dram_parameter("id16", [128, 128], F16, isOutput=False)
    id32_d = nc.declare_dram_parameter("id32", [128, 128], F32, isOutput=False)
    out_d = nc.declare_dram_parameter("out", [rows, Q * D], F16, isOutput=True)

    keys_ap = keys_d.ap()
    mask_ap = mask_d.ap()
    ctx_ap = ctx_d.ap()
    out_ap = out_d.ap()

    n_blk = (rows + BLK - 1) // BLK

    import contextlib

    with contextlib.ExitStack() as ctx:
        singles = ctx.enter_context(tc.tile_pool(name="singles", bufs=1))
        kpool = ctx.enter_context(tc.tile_pool(name="kpool", bufs=14))
        ktpool = ctx.enter_context(tc.tile_pool(name="ktpool", bufs=3))
        blkpool = ctx.enter_context(tc.tile_pool(name="blkpool", bufs=2))
        qktpool = ctx.enter_context(tc.tile_pool(name="qktpool", bufs=2))
        work = ctx.enter_context(tc.tile_pool(name="work", bufs=3))
        mpool = ctx.enter_context(tc.tile_pool(name="mpool", bufs=3))
        small = ctx.enter_context(tc.tile_pool(name="small", bufs=4))
        ps = ctx.enter_context(tc.tile_pool(name="ps", bufs=1, space="PSUM"))

        n_grp_total = rows // GRP
        PF = 10  # keys DMA prefetch distance (groups); the bank absorbs the
        #          small per-group DMA deficit (keys+out+mask > cadence)
        staged = {}
        mstaged = {}

        def _load_group(g):
            if g >= n_grp_total or g in staged:
                return
            g0 = g * GRP
            # p-major layout: partition p holds rows s = p*NC + c, giving
            # 2 KiB-contiguous DMA read descriptors (c,d) per (p, r).
            # SWDGE (gpsimd) DMA casts f32 -> f16 inline: no Act/DVE cast
            # work and half the SBUF write traffic. The Q7 issues ONLY these
            # so the keys stream is never head-of-line blocked.
            k16 = kpool.tile([128, GRP * NC, D], F16, tag="k16",
                             name=f"k16_{g}")
            nc.gpsimd.dma_start(
                out=k16.rearrange("p (r c) d -> p r (c d)", r=GRP),
                in_=keys_ap[g0 : g0 + GRP].rearrange(
                    "r (p c) d -> p r (c d)", p=128
                ),
            )
            staged[g] = k16

        # ---- constants (loaded once, scalar HWDGE ring; sync keeps out).
        # qkt-critical consts (id32/id16/maug) go FIRST, and the keys
        # prefetch is staggered (2 groups now, rest in the first loop
        # iterations) so the big keys stream doesn't starve these small
        # loads out of the shared SDMA engines during the prologue. ----
        # id16/id32 open the gpsimd ring (ahead of maug + keys): the warmup
        # spin and the first transposes gate on them.
        id16_sb = singles.tile([128, 128], F16)
        nc.gpsimd.dma_start(out=id16_sb, in_=id16_d.ap())
        id32_sb = singles.tile([128, 128], F32)
        nc.gpsimd.dma_start(out=id32_sb, in_=id32_d.ap())
        # maug is 532 KB and gates the qkt prologue; four slice-DMAs give
        # per-slice semaphores so qkt q0 starts as soon as slice 0 lands
        # instead of waiting for the whole tensor behind the keys stream.
        # maug rides the gpsimd (SWDGE) ring AHEAD of the keys loads in Q7
        # program order: it lands in ~2us instead of queueing behind 8 MiB
        # of keys on the shared SDMA engines (measured arrival ~25us when
        # it shared the scalar ring with the keys stream active).
        maug_sb = singles.tile([C + 1, Q, D], F16)
        nc.gpsimd.dma_start(out=maug_sb, in_=maug_d.ap())
        wvt_sb = singles.tile([D, D], F16)
        nc.scalar.dma_start(out=wvt_sb, in_=wvt_d.ap())
        mstat_sb = singles.tile([MP, MB, 128], F16)
        nc.scalar.dma_start(out=mstat_sb, in_=mstat_d.ap())
        etile_sb = singles.tile([MB * GRP, MB, 128], F16)
        nc.scalar.dma_start(out=etile_sb, in_=etile_d.ap())

        # ---- conditioned queries qk'^T, per block (block 1 emitted lazily
        # inside the group loop to overlap with the DMA-bound fill phase) ----
        qkt_blocks = {}
        ctxt_blocks = {}

        def _qkt_prep(blk):
            r0 = blk * BLK
            bn = min(BLK, rows - r0)
            assert bn % GRP == 0
            ctx_sb = blkpool.tile([BLK, C], F32, tag="ctx")
            eng = nc.gpsimd if blk == 0 else nc.scalar
            eng.dma_start(out=ctx_sb[:bn], in_=ctx_ap[r0 : r0 + bn])
            ctxt_ps = ps.tile([C, BLK], F32, tag="smallf32", bufs=2)
            nc.tensor.transpose(ctxt_ps[:, :bn], ctx_sb[:bn], id32_sb[:bn, :bn])
            ctxt_sb = blkpool.tile([C + 1, BLK], F16, tag="ctxt")
            nc.vector.tensor_copy(out=ctxt_sb[:C, :bn], in_=ctxt_ps[:, :bn])
            nc.vector.memset(ctxt_sb[C : C + 1, :bn], 1.0)
            ctxt_blocks[blk] = ctxt_sb
            # q-major layout: the per-q PSUM->SBUF drains write contiguous
            # columns (strided writes measured ~6x slower on DVE)
            qkt_blocks[blk] = qktpool.tile([D, Q, BLK], F16, tag="qkt",
                                           name=f"qkt_{blk}")

        def _qkt_chunk(blk, q_lo, q_hi, split=False):
            """qk'^T cols [q_lo, q_hi): [D, Q, bn] fp16, prescaled.
            split=True alternates the PSUM drains DVE/Act (prologue only,
            when Act is otherwise idle)."""
            r0 = blk * BLK
            bn = min(BLK, rows - r0)
            ctxt_sb = ctxt_blocks[blk]
            qkt_sb = qkt_blocks[blk]
            for q in range(q_lo, q_hi, 2):
                qkt_ps = ps.tile([D, 2, BLK], F32, tag="smallf32", bufs=2)
                nc.tensor.matmul(
                    qkt_ps[:, 0, :bn], maug_sb[:, q, :], ctxt_sb[:, :bn],
                    start=True, stop=True, skip_group_check=True,
                )
                nc.tensor.matmul(
                    qkt_ps[:, 1, :bn], maug_sb[:, q + 1, :], ctxt_sb[:, :bn],
                    start=True, stop=True, skip_group_check=True,
                )
                if split and (q // 2) % 2:
                    nc.scalar.copy(out=qkt_sb[:, q : q + 2, :bn],
                                   in_=qkt_ps[:, :, :bn])
                else:
                    nc.vector.tensor_copy(out=qkt_sb[:, q : q + 2, :bn],
                                          in_=qkt_ps[:, :, :bn])

        def _load_mask_batch(mb):
            """DMA+cast mask for MB groups (permuted to the p-major column
            order s = p*NC + c); also derive n_real-1 per row."""
            if mb >= (n_grp_total + MB - 1) // MB or mb in mstaged:
                return
            m0 = mb * MB * GRP
            mrows = MB * GRP
            assert m0 + mrows <= rows
            masku = mpool.tile([mrows, S], U8, tag="masku")
            nc.sync.dma_start(out=masku, in_=mask_ap[m0 : m0 + mrows])
            # mtile rows 0..31: mask (f16, column-permuted);
            # rows 32,33: svals' and ones (DMA'd consts)
            mtile = mpool.tile([MP, S], F16, tag="mtile")
            nc.vector.tensor_copy(
                out=mtile[:mrows].rearrange("i (c p) -> i c p", c=NC),
                in_=masku.rearrange("i (p c) -> i c p", p=128),
            )
            nc.sync.dma_start(out=mtile[mrows : mrows + 2], in_=svals_d.ap())
            nm1f = mpool.tile([mrows, 1], F32, tag="nm1f")
            nc.vector.tensor_reduce(
                out=nm1f, in_=mtile[:mrows], axis=mybir.AxisListType.X,
                op=mybir.AluOpType.add,
            )
            nm1 = mpool.tile([mrows, 1], F16, tag="nm1")
            nc.vector.tensor_scalar(
                out=nm1, in0=nm1f, scalar1=-1.0, scalar2=None,
                op0=mybir.AluOpType.add,
            )
            mstaged[mb] = (mtile, nm1)

        # PE warmup: the tensor engine p-state needs ~3us of continuous
        # execution to reach 2.4 GHz; these dummy transposes (input: id16
        # only, output never read) ramp the clock while the remaining
        # constants and the first keys groups stream in, so the qkt
        # prologue matmuls run hot instead of at 0.65 GHz.
        warm_ps = ps.tile([128, 128], F16, tag="smallf32", bufs=2)
        for _ in range(36):
            nc.tensor.transpose(warm_ps, id16_sb, id16_sb)

        _qkt_prep(0)
        _load_group(0)
        _load_mask_batch(0)
        _qkt_chunk(0, 0, Q, split=True)
        # rest of the prefetch after the qkt emission so 5 MiB of keys
        # doesn't starve the constant loads out of the SDMA engines
        for g in range(1, PF):
            _load_group(g)

        pend = {}  # g -> (k16, e_sb, rs_sb) handed from front to back stage
        tdone = {}  # g -> (k16, kt_sb): transpose stage, runs 2 groups ahead

        def _front_t(g):
            """keys^T per row via PE transpose; runs TWO groups ahead of
            pass 1 so the serial DVE ktp drains never gate the pass-1
            quadrant burst."""
            if g >= n_grp_total:
                return
            k16 = staged.pop(g)
            kt_sb = ktpool.tile([128, GRP, S], F16, tag="kt",
                                name=f"kt_{g}")
            for r in range(GRP):
                ktp = ps.tile([128, S], F16, tag="ktp", bufs=3)
                for c in range(NC):
                    nc.tensor.transpose(
                        ktp[:, c * 128 : (c + 1) * 128],
                        k16[:, r * NC + c, :], id16_sb,
                    )
                nc.vector.tensor_copy(out=kt_sb[:, r, :], in_=ktp)
            tdone[g] = (k16, kt_sb)

        def _front_b(g):
            """Bias/shift matmuls + pass 1 + softmax."""
            g0 = g * GRP
            qkt_sb = qkt_blocks[g0 // BLK]
            gl = (g0 % BLK) // GRP
            _load_group(g + PF)
            if g % MB == 0:
                _load_mask_batch(g // MB + 1)
            mtile, nm1 = mstaged[g // MB]
            jm = g % MB  # position of this group within the mask batch
            k16, kt_sb = tdone.pop(g)

            # ---- shift: cb[p] = -slope[p%32] * (n_real[r(p)] - 1) ----
            cb_ps = ps.tile([128, 1], F32, tag="smallf32", bufs=2)
            nc.tensor.matmul(
                cb_ps, etile_sb[:, jm, :], nm1,
                start=True, stop=True, tile_position=(0, 0),
            )
            cb_sb = small.tile([128, 1], F32, tag="cb")
            nc.scalar.copy(out=cb_sb, in_=cb_ps)

            # ---- bias+mask matmul opens the logits accumulation ----
            lg_ps = ps.tile([128, S], F32, tag="logits", bufs=2)
            nc.tensor.matmul(
                lg_ps, mstat_sb[:, jm, :], mtile,
                start=True, stop=False,
                skip_group_check=True, tile_position=(0, 0),
            )

            # ---- pass 1: logits += qk'.keys (4 concurrent PE quadrants) ----
            for r in range(GRP):
                nc.tensor.matmul(
                    lg_ps[32 * r : 32 * (r + 1), :],
                    qkt_sb[:, :, gl * GRP + r],
                    kt_sb[:, r, :],
                    start=False, stop=(r == GRP - 1),
                    tile_position=(0, 32 * r),
                    skip_group_check=True,
                )

            # ---- softmax (no max pass): e = exp(lg/SC + cb), sum ----
            e_sb = work.tile([128, S], F16, tag="e")
            sum_sb = small.tile([128, 1], F32, tag="sum")
            nc.scalar.activation(
                out=e_sb, in_=lg_ps, func=mybir.ActivationFunctionType.Exp,
                bias=cb_sb, scale=1.0 / SC, accum_out=sum_sb,
            )
            rs_sb = small.tile([128, 1], F32, tag="rs")
            nc.vector.reciprocal(rs_sb, sum_sb)
            pend[g] = (k16, e_sb, rs_sb)
            if g % MB == MB - 1:
                mstaged.pop(g // MB, None)

        def _back(g):
            """AV stage: w transpose, pass 2, Wv projection, store.
            Emitted one group behind the front stage so the PE fills the
            exp-wait with the next group's transposes instead of stalling."""
            k16, e_sb, rs_sb = pend.pop(g)
            g0 = g * GRP

            # ---- w^T: [s_in_chunk, c, rq] fp16 ----
            wt_ps = ps.tile([128, NC, 128], F16, tag="wtps", bufs=1)
            for c in range(NC):
                nc.tensor.transpose(
                    wt_ps[:, c, :], e_sb[:, c * 128 : (c + 1) * 128], id16_sb
                )
            wt_sb = work.tile([128, NC, 128], F16, tag="wt")
            nc.vector.tensor_copy(out=wt_sb, in_=wt_ps)

            # ---- pass 2: pk^T[d, rq] = sum_s keys[s,d] * w[rq,s] ----
            pk_ps = ps.tile([128, 128], F32, tag="smallf32", bufs=2)
            for r in range(GRP):
                for c in range(NC):
                    nc.tensor.matmul(
                        pk_ps[:, 32 * r : 32 * (r + 1)],
                        k16[:, r * NC + c, :],
                        wt_sb[:, c, 32 * r : 32 * (r + 1)],
                        start=(c == 0), stop=(c == NC - 1),
                        skip_group_check=True,
                    )
            pkt_sb = work.tile([128, 128], F16, tag="pkt")
            nc.scalar.copy(out=pkt_sb, in_=pk_ps)

            # ---- pooled[rq, e] = pk^T.T @ Wv^T ----
            po_ps = ps.tile([128, 128], F32, tag="smallf32", bufs=2)
            nc.tensor.matmul(po_ps, pkt_sb, wvt_sb, start=True, stop=True)

            o_sb = work.tile([128, 128], F16, tag="o")
            nc.scalar.activation(
                out=o_sb, in_=po_ps, func=mybir.ActivationFunctionType.Copy,
                scale=rs_sb,
            )
            nc.sync.dma_start(
                out=out_ap[g0 : g0 + GRP].rearrange("r (q e) -> (r q) e", e=D),
                in_=o_sb,
            )

        # block-1 qkt is emitted in 4-query chunks during the first loop
        # iterations: the fill phase is DMA-bound, so the PE absorbs it
        # nearly free.
        _front_t(0)
        _front_t(1)
        for g in range(n_grp_total):
            _front_b(g)
            _front_t(g + 2)
            if g > 0:
                _back(g - 1)
            if n_blk > 1:
                if g == 0:
                    _qkt_prep(1)
                if 1 <= g <= 16:
                    _qkt_chunk(1, (g - 1) * 2, g * 2)
        _back(n_grp_total - 1)


def _build(rows):
    nc = bacc.Bacc(trn_type="TRN2", target_bir_lowering=False, debug=False)
    with tile.TileContext(nc) as tc:
        _emit(nc, tc, rows)
    nc.compile()
    return nc


def host_consts(queries, Wk, log_temperature, Wc, bc, Wv):
    """Fold projections/scales into small host-side constants."""
    queries = np.asarray(queries, np.float64)
    Wk = np.asarray(Wk, np.float64)
    Wc = np.asarray(Wc, np.float64)
    bc = np.asarray(bc, np.float64)
    Wv = np.asarray(Wv, np.float64)
    lt = np.asarray(log_temperature, np.float64)

    scale = D ** -0.5
    inv_t = np.repeat(np.exp(-lt), H)  # [Q]
    slopes_h = 2.0 ** (-8.0 * (np.arange(H) + 1) / H)
    slopes = np.tile(slopes_h, T)  # [Q]
    s_q = scale * inv_t  # [Q]

    q_eff = queries.reshape(Q, D) + bc.reshape(Q, D)  # [Q, D]
    qk0 = q_eff @ Wk  # [Q, D]
    # maug[c, q, d]: rows 0..C-1 = SC*s_q * (Wc_q^T @ Wk); row C = SC*s_q * qk0
    maug = np.empty((C + 1, Q, D), np.float64)
    for q in range(Q):
        Wc_q = Wc[q * D : (q + 1) * D, :]  # [D(e), C]
        maug[:C, q, :] = (Wc_q.T @ Wk) * (SC * s_q[q])
        maug[C, q, :] = qk0[q] * (SC * s_q[q])

    slope_col = np.tile(slopes, 128 // Q)  # [128], p -> slopes[p % 32]

    # mstat[m]: bias matmul weights vs the extended [34, S] mask tile.
    # Rows 4m..4m+3 select this group's mask rows (+MASK_NEG on unmasked),
    # row 32 pairs with svals' (SC*slope[p]*s), row 33 with ones (-MASK_NEG).
    MP = MB * GRP + 2
    mstat = np.zeros((MB, MP, 128), np.float16)
    for m in range(MB):
        for r in range(GRP):
            mstat[m, GRP * m + r, 32 * r : 32 * (r + 1)] = MASK_NEG
        mstat[m, MB * GRP] = (SC * slope_col).astype(np.float16)
        mstat[m, MB * GRP + 1] = np.float16(-MASK_NEG)

    # svals row 0: s-value at logits column j under the p-major layout
    # (j = c*128 + p  <->  s = p*NC + c); row 1: ones. All exact fp16.
    NCh = S // 128
    j = np.arange(S)
    svals = np.zeros((2, S), np.float16)
    svals[0] = ((j % 128) * NCh + j // 128).astype(np.float16)
    svals[1] = np.float16(1.0)

    # etile[m]: cb[p] = sum_i etile[m, i, p]*(n_i - 1) = -slope[p]*(n-1)
    etile = np.zeros((MB, MB * GRP, 128), np.float16)
    for m in range(MB):
        for r in range(GRP):
            etile[m, GRP * m + r, 32 * r : 32 * (r + 1)] = -slopes.astype(
                np.float16
            )

    return dict(
        maug=maug.astype(np.float16),
        wvt=np.ascontiguousarray(Wv.T).astype(np.float16),
        # shipped pre-transposed so the device DMA is contiguous (the
        # rearranged load generated 256-B descriptors and ~8 us of ring
        # backpressure during the prologue)
        mstat=np.ascontiguousarray(mstat.transpose(1, 0, 2)),
        svals=svals,
        etile=np.ascontiguousarray(etile.transpose(1, 0, 2)),
        id16=np.eye(128, dtype=np.float16),
        id32=np.eye(128, dtype=np.float32),
    )


def make_in_maps(keys, mask, context, consts, rows, n_cores):
    keys = np.asarray(keys, np.float32)
    mask_u8 = np.asarray(mask).astype(np.uint8)
    ctx = np.asarray(context, np.float32)
    in_maps = []
    for i in range(n_cores):
        sl = slice(i * rows, (i + 1) * rows)
        in_maps.append(
            dict(
                keys=np.ascontiguousarray(keys[sl]),
                mask=np.ascontiguousarray(mask_u8[sl]),
                ctx=np.ascontiguousarray(ctx[sl]),
                **consts,
            )
        )
    return in_maps


_cache = {}


def run(keys, mask, context, queries, Wk, Wv, log_temperature, Wc, bc,
        trace=False, **kw):
    consts = host_consts(queries, Wk, log_temperature, Wc, bc, Wv)
    if ROWS not in _cache:
        _cache[ROWS] = _build(ROWS)
    nc = _cache[ROWS]
    in_maps = make_in_maps(keys, mask, context, consts, ROWS, N_CORES)
    res = run_bass_kernel_spmd(nc, in_maps, core_ids=list(range(N_CORES)),
                               trace=trace, **kw)
    out = np.concatenate([res.results[i]["out"] for i in range(N_CORES)], axis=0)
    return out.reshape(B, T, H * D).astype(np.float32), res


def kernel(keys, mask, context, queries, Wk, Wv, log_temperature, Wc, bc):
    out, _ = run(keys, mask, context, queries, Wk, Wv, log_temperature, Wc, bc)
    return out

